# revision 2
# baseline (speedup 1.0000x reference)
"""CosHead kernel for Trainium2 (8 NeuronCores, Bass/Tile).

out[c, h, w] = cos_sim(x[:, h, w], weights[c]) * scale[c] * 5.0

Sharding: spatial (H) split across the 8 cores — each core reads only its
1/8 slice of x (8.4 MB) and writes its 1/8 slice of the output, which is the
minimum possible HBM traffic (the sharding hint's class-split would replicate
all 67 MB of x onto every core).

Per-core device pipeline (npix = 8192 pixels, D = 256 latent, C = 256 classes):
  - DMA in x as two partition chunks [128, npix] (D on partitions).
  - ACT: xsq = x^2 (bf16 out — feeds only the norm reduction).
  - PE:  norm2 = ones[128,128].T @ xsq (bf16 matmul, accumulated over the two
         D chunks) -> PSUM tile whose 128 rows all equal the per-pixel
         sum-of-squares broadcast.
  - PE:  y = wfoldT.T @ x  (fp32r matmuls — full PE rate vs 1/4 rate for
         plain fp32 — accumulated over D chunks), where
         wfoldT[d, c] = weights.T * (5 * scale[c] / max(||w_c||, eps)) is
         folded on the host (O(C*D) work).
  - ACT: norm = sqrt(norm2);  DVE: inv = 1/norm;  DVE: out = y * inv.
  - DMA out [128, npix] per class chunk.

x and wt are declared float32r end-to-end (DRAM + SBUF); the host supplies
raw fp32 bits. The PE's fp32r path applies its internal rounding when
consuming them; the ACT square reads the same bytes bitcast back to fp32.

The weight normalization + scale fold + transpose is O(C*D) = 65K elements
(0.001% of the 8.6 GFLOP) and is done on the host; all O(H*W*D) work runs on
the device.
"""

import numpy as np
from contextlib import ExitStack

import concourse.bacc as bacc
import concourse.tile as tile
from concourse import mybir
from concourse.bass_utils import run_bass_kernel_spmd

N_CORES = 8
C = 256           # n_classes
D = 256           # latent
H = 256
W = 256
HL = H // N_CORES # 32 rows of H per core
NPIX = HL * W     # 8192 pixels per core
EPS = 1e-8
RANGE_EXTENDER = 5.0

STAGE = 2048      # pixels per pipeline stage (1 MB DMA per chunk per stage)
PT = 512          # pixels per PSUM tile (one fp32 bank; fp32 moving-op max)

F32 = mybir.dt.float32
F32R = mybir.dt.float32r
BF16 = mybir.dt.bfloat16

_CACHE = {}


def build(repeat=1, mm_dt=F32R, stage=STAGE, pt=PT, npix=NPIX):
    """Build + compile the SPMD per-core program. repeat>1 wraps the whole
    pipeline in a hardware loop (for timing measurements)."""
    nc = bacc.Bacc("TRN2", target_bir_lowering=False, debug=False)
    x_t = nc.dram_tensor("x", [2, 128, npix], mm_dt, kind="ExternalInput")
    w_t = nc.dram_tensor("wt", [2, 128, C], mm_dt, kind="ExternalInput")
    o_t = nc.dram_tensor("out", [2, 128, npix], F32, kind="ExternalOutput")
    x_d, w_d, o_d = x_t.ap(), w_t.ap(), o_t.ap()

    with ExitStack() as ctx:
        tc = ctx.enter_context(tile.TileContext(nc))
        consts = ctx.enter_context(tc.tile_pool(name="consts", bufs=1))
        xp = ctx.enter_context(tc.tile_pool(name="xp", bufs=2))
        qp = ctx.enter_context(tc.tile_pool(name="qp", bufs=2))
        op = ctx.enter_context(tc.tile_pool(name="op", bufs=2))
        vp = ctx.enter_context(tc.tile_pool(name="vp", bufs=2))
        pp = ctx.enter_context(tc.tile_pool(name="pp", bufs=2, space="PSUM"))

        w0 = consts.tile([128, C], mm_dt)
        nc.sync.dma_start(w0[:], w_d[0])
        w1 = consts.tile([128, C], mm_dt)
        nc.sync.dma_start(w1[:], w_d[1])
        ones = consts.tile([128, 128], BF16)
        nc.vector.memset(ones[:], 1.0)

        def body():
            for s in range(npix // stage):
                c0 = s * stage
                x0 = xp.tile([128, stage], mm_dt, tag="x0")
                nc.sync.dma_start(x0[:], x_d[0, :, c0:c0 + stage])
                x1 = xp.tile([128, stage], mm_dt, tag="x1")
                nc.sync.dma_start(x1[:], x_d[1, :, c0:c0 + stage])
                q0 = qp.tile([128, stage], BF16, tag="q0")
                nc.scalar.activation(q0[:], x0[:].bitcast(F32),
                                     mybir.ActivationFunctionType.Square)
                q1 = qp.tile([128, stage], BF16, tag="q1")
                nc.scalar.activation(q1[:], x1[:].bitcast(F32),
                                     mybir.ActivationFunctionType.Square)
                o0 = op.tile([128, stage], F32, tag="o0")
                o1 = op.tile([128, stage], F32, tag="o1")
                for t in range(stage // pt):
                    sl = slice(t * pt, (t + 1) * pt)
                    pn = pp.tile([128, pt], F32, tag="pn")
                    nc.tensor.matmul(pn[:], ones[:], q0[:, sl],
                                     start=True, stop=False)
                    nc.tensor.matmul(pn[:], ones[:], q1[:, sl],
                                     start=False, stop=True)
                    p0 = pp.tile([128, pt], F32, tag="p0")
                    nc.tensor.matmul(p0[:], w0[:, 0:128], x0[:, sl],
                                     start=True, stop=False)
                    nc.tensor.matmul(p0[:], w1[:, 0:128], x1[:, sl],
                                     start=False, stop=True)
                    p1 = pp.tile([128, pt], F32, tag="p1")
                    nc.tensor.matmul(p1[:], w0[:, 128:256], x0[:, sl],
                                     start=True, stop=False)
                    nc.tensor.matmul(p1[:], w1[:, 128:256], x1[:, sl],
                                     start=False, stop=True)
                    nrm = vp.tile([128, pt], F32, tag="nrm")
                    nc.scalar.activation(nrm[:], pn[:],
                                         mybir.ActivationFunctionType.Sqrt)
                    inv = vp.tile([128, pt], F32, tag="inv")
                    nc.vector.reciprocal(inv[:], nrm[:])
                    nc.vector.tensor_mul(o0[:, sl], p0[:], inv[:])
                    nc.vector.tensor_mul(o1[:, sl], p1[:], inv[:])
                nc.sync.dma_start(o_d[0, :, c0:c0 + stage], o0[:])
                nc.sync.dma_start(o_d[1, :, c0:c0 + stage], o1[:])

        if repeat == 1:
            body()
        else:
            with tc.For_i(0, repeat, 1):
                body()

    nc.compile()
    return nc


def _get_prog():
    key = "main"
    if key not in _CACHE:
        _CACHE[key] = build()
    return _CACHE[key]


def prep_inputs(x, weights, scale):
    """Host-side prep: shard x spatially, fold norm+scale into transposed
    weights. Returns in_maps for the 8 cores."""
    x = np.ascontiguousarray(np.asarray(x, dtype=np.float32))
    weights = np.asarray(weights, dtype=np.float32)
    scale = np.asarray(scale, dtype=np.float32)

    wnorm = np.sqrt((weights * weights).sum(axis=1))
    sfold = (RANGE_EXTENDER * scale) / np.maximum(wnorm, EPS)
    wT = np.ascontiguousarray((weights * sfold[:, None]).T.astype(np.float32))
    wT = wT.reshape(2, 128, C)

    in_maps = []
    for k in range(N_CORES):
        xl = np.ascontiguousarray(x[:, k * HL:(k + 1) * HL, :])
        in_maps.append({"x": xl.reshape(2, 128, NPIX), "wt": wT})
    return in_maps


def gather_output(results):
    outs = [res["out"].reshape(C, HL, W) for res in results]
    return np.concatenate(outs, axis=1)


def kernel(x, weights, scale):
    in_maps = prep_inputs(x, weights, scale)
    nc = _get_prog()
    res = run_bass_kernel_spmd(nc, in_maps, core_ids=list(range(N_CORES)))
    return gather_output(res.results)


# revision 19
# speedup vs baseline: 1.8024x; 1.8024x over previous
"""CosHead kernel for Trainium2 (8 NeuronCores, Bass/Tile).

out[c, h, w] = cos_sim(x[:, h, w], weights[c]) * scale[c] * 5.0

Sharding: spatial (H) split across the 8 cores — each core reads only its
1/8 slice of x (8.4 MB) and writes its 1/8 slice of the output, which is the
minimum possible HBM traffic (the sharding hint's class-split would replicate
all 67 MB of x onto every core).

Per-core device pipeline (npix = 8192 pixels, D = 256 latent, C = 256 classes):
  - DMA in x as two partition chunks [128, npix] (D on partitions).
  - ACT: xsq = x^2 (bf16 out — feeds only the norm reduction).
  - PE:  norm2 = ones[128,128].T @ xsq (bf16 matmul, accumulated over the two
         D chunks) -> PSUM tile whose 128 rows all equal the per-pixel
         sum-of-squares broadcast.
  - PE:  y = wfoldT.T @ x  (fp32r matmuls — full PE rate vs 1/4 rate for
         plain fp32 — accumulated over D chunks), where
         wfoldT[d, c] = weights.T * (5 * scale[c] / max(||w_c||, eps)) is
         folded on the host (O(C*D) work).
  - ACT: norm = sqrt(norm2);  DVE: inv = reciprocal_approx_fast(norm)
    (single custom-DVE op, ~18 correct bits — the standard
    nc.vector.reciprocal is ~5x slower and was the critical path);
    DVE: out = y * inv.
  - DMA out [128, npix] per class chunk, issued from the scalar engine's
    HWDGE ring so output DMAs don't head-of-line-block input DMAs on the
    sync ring.

Measured on HW (repeat-loop slope method): ~53 us/pass, which matches the
pure-DMA ablation of the same transfers (~52 us) — the kernel is at the HBM
bandwidth roofline (16.8 MB/core at ~320 GB/s/core across 8 cores).

x and wt are declared float32r end-to-end (DRAM + SBUF); the host supplies
raw fp32 bits. The PE's fp32r path applies its internal rounding when
consuming them; the ACT square reads the same bytes bitcast back to fp32.

The weight normalization + scale fold + transpose is O(C*D) = 65K elements
(0.001% of the 8.6 GFLOP) and is done on the host; all O(H*W*D) work runs on
the device.
"""

import numpy as np
from contextlib import ExitStack

import concourse.bacc as bacc
import concourse.tile as tile
from concourse import mybir
from concourse.bass_utils import run_bass_kernel_spmd

N_CORES = 8
C = 256           # n_classes
D = 256           # latent
H = 256
W = 256
HL = H // N_CORES # 32 rows of H per core
NPIX = HL * W     # 8192 pixels per core
EPS = 1e-8
RANGE_EXTENDER = 5.0

STAGE = 2048      # pixels per pipeline stage (1 MB DMA per chunk per stage)
PT = 512          # pixels per PSUM tile (one fp32 bank; fp32 moving-op max)

F32 = mybir.dt.float32
F32R = mybir.dt.float32r
BF16 = mybir.dt.bfloat16

_CACHE = {}


def build(repeat=1, mm_dt=F32R, stage=STAGE, pt=PT, npix=NPIX,
          bufs=3, out_split=None, staggered=False, dma_only=False,
          out_engine="scalar", mode="full"):
    """Build + compile the SPMD per-core program. repeat>1 wraps the whole
    pipeline in a hardware loop (for timing measurements)."""
    nc = bacc.Bacc("TRN2", target_bir_lowering=False, debug=False)
    x_t = nc.dram_tensor("x", [2, 128, npix], mm_dt, kind="ExternalInput")
    w_t = nc.dram_tensor("wt", [2, 128, C], mm_dt, kind="ExternalInput")
    o_t = nc.dram_tensor("out", [2, 128, npix], F32, kind="ExternalOutput")
    x_d, w_d, o_d = x_t.ap(), w_t.ap(), o_t.ap()
    if out_split is None:
        out_split = stage
    out_eng = {"sync": "sync", "scalar": "scalar", "gpsimd": "gpsimd"}[out_engine]

    with ExitStack() as ctx:
        tc = ctx.enter_context(tile.TileContext(nc))
        consts = ctx.enter_context(tc.tile_pool(name="consts", bufs=1))
        xp = ctx.enter_context(tc.tile_pool(name="xp", bufs=bufs))
        qp = ctx.enter_context(tc.tile_pool(name="qp", bufs=bufs))
        op = ctx.enter_context(tc.tile_pool(name="op", bufs=bufs))
        vp_bufs = 2 * (stage // pt) if mode == "pipe" else bufs
        vp = ctx.enter_context(tc.tile_pool(name="vp", bufs=vp_bufs))
        pp = ctx.enter_context(tc.tile_pool(name="pp", bufs=2, space="PSUM"))
        if mode == "full2":
            ppn = ctx.enter_context(tc.tile_pool(name="ppn", bufs=1, space="PSUM"))
        if mode == "pipe":
            ppn = ctx.enter_context(tc.tile_pool(name="ppn", bufs=4, space="PSUM"))

        w0 = consts.tile([128, C], mm_dt)
        nc.sync.dma_start(w0[:], w_d[0])
        w1 = consts.tile([128, C], mm_dt)
        nc.sync.dma_start(w1[:], w_d[1])
        ones = consts.tile([128, 128], BF16)
        nc.vector.memset(ones[:], 1.0)

        def body(mode="full"):
            if dma_only:
                for s in range(npix // stage):
                    c0 = s * stage
                    x0 = xp.tile([128, stage], mm_dt, tag="x0")
                    nc.sync.dma_start(x0[:], x_d[0, :, c0:c0 + stage])
                    x1 = xp.tile([128, stage], mm_dt, tag="x1")
                    nc.sync.dma_start(x1[:], x_d[1, :, c0:c0 + stage])
                    o0 = op.tile([128, stage], F32, tag="o0")
                    nc.vector.tensor_copy(o0[:, 0:1], x0[:, 0:1])
                    o1 = op.tile([128, stage], F32, tag="o1")
                    nc.vector.tensor_copy(o1[:, 0:1], x1[:, 0:1])
                    getattr(nc, out_eng).dma_start(o_d[0, :, c0:c0 + stage], o0[:])
                    getattr(nc, out_eng).dma_start(o_d[1, :, c0:c0 + stage], o1[:])
                return
            if mode == "pipe":
                nstages = npix // stage
                nt = stage // pt

                def norm_chain(s):
                    """in-DMA + squares + norm matmuls + sqrt + recip for
                    stage s; returns (x0, x1, inv_tiles)."""
                    c0 = s * stage
                    x0 = xp.tile([128, stage], mm_dt, tag="x0")
                    nc.sync.dma_start(x0[:], x_d[0, :, c0:c0 + stage])
                    x1 = xp.tile([128, stage], mm_dt, tag="x1")
                    nc.sync.dma_start(x1[:], x_d[1, :, c0:c0 + stage])
                    q0 = qp.tile([128, stage], BF16, tag="q0")
                    nc.scalar.activation(q0[:], x0[:].bitcast(F32),
                                         mybir.ActivationFunctionType.Square)
                    q1 = qp.tile([128, stage], BF16, tag="q1")
                    nc.scalar.activation(q1[:], x1[:].bitcast(F32),
                                         mybir.ActivationFunctionType.Square)
                    invs = []
                    for t in range(nt):
                        sl = slice(t * pt, (t + 1) * pt)
                        pn = ppn.tile([128, pt], F32, tag="pn")
                        nc.tensor.matmul(pn[:], ones[:], q0[:, sl],
                                         start=True, stop=False)
                        nc.tensor.matmul(pn[:], ones[:], q1[:, sl],
                                         start=False, stop=True)
                        nrm = vp.tile([128, pt], F32, tag="nrm")
                        nc.scalar.activation(nrm[:], pn[:],
                                             mybir.ActivationFunctionType.Sqrt)
                        inv = vp.tile([128, pt], F32, tag="inv")
                        nc.vector.reciprocal_approx_fast(inv[:], nrm[:])
                        invs.append(inv)
                    return x0, x1, invs

                def main_stage(s, x0, x1, invs):
                    c0 = s * stage
                    o0 = op.tile([128, stage], F32, tag="o0")
                    o1 = op.tile([128, stage], F32, tag="o1")
                    for t in range(nt):
                        sl = slice(t * pt, (t + 1) * pt)
                        p0 = pp.tile([128, pt], F32, tag="p0")
                        nc.tensor.matmul(p0[:], w0[:, 0:128], x0[:, sl],
                                         start=True, stop=False)
                        nc.tensor.matmul(p0[:], w1[:, 0:128], x1[:, sl],
                                         start=False, stop=True)
                        p1 = pp.tile([128, pt], F32, tag="p1")
                        nc.tensor.matmul(p1[:], w0[:, 128:256], x0[:, sl],
                                         start=True, stop=False)
                        nc.tensor.matmul(p1[:], w1[:, 128:256], x1[:, sl],
                                         start=False, stop=True)
                        nc.vector.tensor_mul(o0[:, sl], p0[:], invs[t][:])
                        nc.vector.tensor_mul(o1[:, sl], p1[:], invs[t][:])
                    for u0 in range(0, stage, out_split):
                        getattr(nc, out_eng).dma_start(
                            o_d[0, :, c0 + u0:c0 + u0 + out_split],
                            o0[:, u0:u0 + out_split])
                        getattr(nc, out_eng).dma_start(
                            o_d[1, :, c0 + u0:c0 + u0 + out_split],
                            o1[:, u0:u0 + out_split])

                prev = norm_chain(0)
                for s in range(nstages):
                    nxt = norm_chain(s + 1) if s + 1 < nstages else None
                    main_stage(s, *prev)
                    prev = nxt
                return
            if mode == "full2":
                for s in range(npix // stage):
                    c0 = s * stage
                    x0 = xp.tile([128, stage], mm_dt, tag="x0")
                    nc.sync.dma_start(x0[:], x_d[0, :, c0:c0 + stage])
                    x1 = xp.tile([128, stage], mm_dt, tag="x1")
                    nc.sync.dma_start(x1[:], x_d[1, :, c0:c0 + stage])
                    q0 = qp.tile([128, stage], BF16, tag="q0")
                    nc.scalar.activation(q0[:], x0[:].bitcast(F32),
                                         mybir.ActivationFunctionType.Square)
                    q1 = qp.tile([128, stage], BF16, tag="q1")
                    nc.scalar.activation(q1[:], x1[:].bitcast(F32),
                                         mybir.ActivationFunctionType.Square)
                    # stage-granular norm: one 4-bank PSUM strip, then one
                    # sqrt + one reciprocal for the whole stage
                    pn = ppn.tile([128, stage], F32, tag="pn")
                    for t in range(stage // pt):
                        sl = slice(t * pt, (t + 1) * pt)
                        nc.tensor.matmul(pn[:, sl], ones[:], q0[:, sl],
                                         start=True, stop=False)
                        nc.tensor.matmul(pn[:, sl], ones[:], q1[:, sl],
                                         start=False, stop=True)
                    nrm = vp.tile([128, stage], F32, tag="nrm")
                    nc.scalar.activation(nrm[:], pn[:],
                                         mybir.ActivationFunctionType.Sqrt)
                    inv = vp.tile([128, stage], F32, tag="inv")
                    nc.vector.reciprocal_approx_fast(inv[:], nrm[:])
                    o0 = op.tile([128, stage], F32, tag="o0")
                    o1 = op.tile([128, stage], F32, tag="o1")
                    for t in range(stage // pt):
                        sl = slice(t * pt, (t + 1) * pt)
                        p0 = pp.tile([128, pt], F32, tag="p0")
                        nc.tensor.matmul(p0[:], w0[:, 0:128], x0[:, sl],
                                         start=True, stop=False)
                        nc.tensor.matmul(p0[:], w1[:, 0:128], x1[:, sl],
                                         start=False, stop=True)
                        p1 = pp.tile([128, pt], F32, tag="p1")
                        nc.tensor.matmul(p1[:], w0[:, 128:256], x0[:, sl],
                                         start=True, stop=False)
                        nc.tensor.matmul(p1[:], w1[:, 128:256], x1[:, sl],
                                         start=False, stop=True)
                        nc.vector.tensor_mul(o0[:, sl], p0[:], inv[:, sl])
                        nc.vector.tensor_mul(o1[:, sl], p1[:], inv[:, sl])
                    for u0 in range(0, stage, out_split):
                        getattr(nc, out_eng).dma_start(
                            o_d[0, :, c0 + u0:c0 + u0 + out_split],
                            o0[:, u0:u0 + out_split])
                        getattr(nc, out_eng).dma_start(
                            o_d[1, :, c0 + u0:c0 + u0 + out_split],
                            o1[:, u0:u0 + out_split])
                return
            do_sq = mode in ("full", "mmq", "mmqn", "mmqns")
            do_pn = mode in ("full", "mmqn", "mmqns")
            do_sqrt = mode in ("full", "mmqns")
            do_recip = mode == "full"
            for s in range(npix // stage):
                c0 = s * stage
                x0 = xp.tile([128, stage], mm_dt, tag="x0")
                nc.sync.dma_start(x0[:], x_d[0, :, c0:c0 + stage])
                x1 = xp.tile([128, stage], mm_dt, tag="x1")
                nc.sync.dma_start(x1[:], x_d[1, :, c0:c0 + stage])
                if do_sq:
                    q0 = qp.tile([128, stage], BF16, tag="q0")
                    nc.scalar.activation(q0[:], x0[:].bitcast(F32),
                                         mybir.ActivationFunctionType.Square)
                    q1 = qp.tile([128, stage], BF16, tag="q1")
                    nc.scalar.activation(q1[:], x1[:].bitcast(F32),
                                         mybir.ActivationFunctionType.Square)
                o0 = op.tile([128, stage], F32, tag="o0")
                o1 = op.tile([128, stage], F32, tag="o1")
                for t in range(stage // pt):
                    sl = slice(t * pt, (t + 1) * pt)
                    if do_pn:
                        pn = pp.tile([128, pt], F32, tag="pn")
                        nc.tensor.matmul(pn[:], ones[:], q0[:, sl],
                                         start=True, stop=False)
                        nc.tensor.matmul(pn[:], ones[:], q1[:, sl],
                                         start=False, stop=True)
                    p0 = pp.tile([128, pt], F32, tag="p0")
                    nc.tensor.matmul(p0[:], w0[:, 0:128], x0[:, sl],
                                     start=True, stop=False)
                    nc.tensor.matmul(p0[:], w1[:, 0:128], x1[:, sl],
                                     start=False, stop=True)
                    p1 = pp.tile([128, pt], F32, tag="p1")
                    nc.tensor.matmul(p1[:], w0[:, 128:256], x0[:, sl],
                                     start=True, stop=False)
                    nc.tensor.matmul(p1[:], w1[:, 128:256], x1[:, sl],
                                     start=False, stop=True)
                    if do_sqrt:
                        nrm = vp.tile([128, pt], F32, tag="nrm")
                        nc.scalar.activation(nrm[:], pn[:],
                                             mybir.ActivationFunctionType.Sqrt)
                    if do_recip:
                        inv = vp.tile([128, pt], F32, tag="inv")
                        nc.vector.reciprocal_approx_fast(inv[:], nrm[:])
                        nc.vector.tensor_mul(o0[:, sl], p0[:], inv[:])
                        nc.vector.tensor_mul(o1[:, sl], p1[:], inv[:])
                    elif do_sqrt:
                        nc.vector.tensor_mul(o0[:, sl], p0[:], nrm[:])
                        nc.vector.tensor_mul(o1[:, sl], p1[:], nrm[:])
                    else:
                        nc.vector.tensor_copy(o0[:, sl], p0[:])
                        nc.vector.tensor_copy(o1[:, sl], p1[:])
                for u0 in range(0, stage, out_split):
                    getattr(nc, out_eng).dma_start(
                        o_d[0, :, c0 + u0:c0 + u0 + out_split],
                        o0[:, u0:u0 + out_split])
                    getattr(nc, out_eng).dma_start(
                        o_d[1, :, c0 + u0:c0 + u0 + out_split],
                        o1[:, u0:u0 + out_split])

        if repeat == 1:
            body(mode)
        else:
            with tc.For_i(0, repeat, 1, staggered_reset=staggered):
                body(mode)

    nc.compile()
    return nc


def _get_prog():
    key = "main"
    if key not in _CACHE:
        _CACHE[key] = build()
    return _CACHE[key]


def prep_inputs(x, weights, scale):
    """Host-side prep: shard x spatially, fold norm+scale into transposed
    weights. Returns in_maps for the 8 cores."""
    x = np.ascontiguousarray(np.asarray(x, dtype=np.float32))
    weights = np.asarray(weights, dtype=np.float32)
    scale = np.asarray(scale, dtype=np.float32)

    wnorm = np.sqrt((weights * weights).sum(axis=1))
    sfold = (RANGE_EXTENDER * scale) / np.maximum(wnorm, EPS)
    wT = np.ascontiguousarray((weights * sfold[:, None]).T.astype(np.float32))
    wT = wT.reshape(2, 128, C)

    in_maps = []
    for k in range(N_CORES):
        xl = np.ascontiguousarray(x[:, k * HL:(k + 1) * HL, :])
        in_maps.append({"x": xl.reshape(2, 128, NPIX), "wt": wT})
    return in_maps


def gather_output(results):
    outs = [res["out"].reshape(C, HL, W) for res in results]
    return np.concatenate(outs, axis=1)


def kernel(x, weights, scale):
    in_maps = prep_inputs(x, weights, scale)
    nc = _get_prog()
    res = run_bass_kernel_spmd(nc, in_maps, core_ids=list(range(N_CORES)))
    return gather_output(res.results)


# revision 24
# speedup vs baseline: 1.8306x; 1.0157x over previous
"""CosHead kernel for Trainium2 (8 NeuronCores, Bass/Tile).

out[c, h, w] = cos_sim(x[:, h, w], weights[c]) * scale[c] * 5.0

Sharding: spatial (H) split across the 8 cores — each core reads only its
1/8 slice of x (8.4 MB) and writes its 1/8 slice of the output, which is the
minimum possible HBM traffic (the sharding hint's class-split would replicate
all 67 MB of x onto every core).

Per-core device pipeline (npix = 8192 pixels, D = 256 latent, C = 256 classes):
  - DMA in x as two partition chunks [128, npix] (D on partitions).
  - ACT: xsq = x^2 (bf16 out — feeds only the norm reduction).
  - PE:  norm2 = ones[128,128].T @ xsq (bf16 matmul, accumulated over the two
         D chunks) -> PSUM tile whose 128 rows all equal the per-pixel
         sum-of-squares broadcast.
  - PE:  y = wfoldT.T @ x  (fp32r matmuls — full PE rate vs 1/4 rate for
         plain fp32 — accumulated over D chunks), where
         wfoldT[d, c] = weights.T * (5 * scale[c] / max(||w_c||, eps)) is
         folded on the host (O(C*D) work).
  - ACT: norm = sqrt(norm2);  DVE: inv = reciprocal_approx_fast(norm)
    (single custom-DVE op, ~18 correct bits — the standard
    nc.vector.reciprocal is ~5x slower and was the critical path);
    DVE: out = y * inv.
  - DMA out [128, npix] per class chunk, issued from the scalar engine's
    HWDGE ring so output DMAs don't head-of-line-block input DMAs on the
    sync ring.

Measured on HW (repeat-loop slope method): ~53 us/pass, which matches the
pure-DMA ablation of the same transfers (~52 us) — the kernel is at the HBM
bandwidth roofline (16.8 MB/core at ~320 GB/s/core across 8 cores).

x and wt are declared float32r end-to-end (DRAM + SBUF); the host supplies
raw fp32 bits. The PE's fp32r path applies its internal rounding when
consuming them; the ACT square reads the same bytes bitcast back to fp32.

The weight normalization + scale fold + transpose is O(C*D) = 65K elements
(0.001% of the 8.6 GFLOP) and is done on the host; all O(H*W*D) work runs on
the device.
"""

import numpy as np
from contextlib import ExitStack

import concourse.bacc as bacc
import concourse.tile as tile
from concourse import mybir
from concourse.bass_utils import run_bass_kernel_spmd

N_CORES = 8
C = 256           # n_classes
D = 256           # latent
H = 256
W = 256
HL = H // N_CORES # 32 rows of H per core
NPIX = HL * W     # 8192 pixels per core
EPS = 1e-8
RANGE_EXTENDER = 5.0

STAGE = 2048      # pixels per pipeline stage (1 MB DMA per chunk per stage)
PT = 512          # pixels per PSUM tile (one fp32 bank; fp32 moving-op max)

F32 = mybir.dt.float32
F32R = mybir.dt.float32r
BF16 = mybir.dt.bfloat16

_CACHE = {}


def build(repeat=1, mm_dt=F32R, stage=STAGE, pt=PT, npix=NPIX,
          bufs=3, out_split=None, staggered=False, dma_only=False,
          out_engine="scalar", mode="full", in2=None, psum3=False):
    """Build + compile the SPMD per-core program. repeat>1 wraps the whole
    pipeline in a hardware loop (for timing measurements)."""
    nc = bacc.Bacc("TRN2", target_bir_lowering=False, debug=False)
    x_t = nc.dram_tensor("x", [2, 128, npix], mm_dt, kind="ExternalInput")
    w_t = nc.dram_tensor("wt", [2, 128, C], mm_dt, kind="ExternalInput")
    o_t = nc.dram_tensor("out", [2, 128, npix], F32, kind="ExternalOutput")
    x_d, w_d, o_d = x_t.ap(), w_t.ap(), o_t.ap()
    if out_split is None:
        out_split = stage
    out_eng = {"sync": "sync", "scalar": "scalar", "gpsimd": "gpsimd"}[out_engine]
    in_eng2 = in2 or "sync"

    with ExitStack() as ctx:
        tc = ctx.enter_context(tile.TileContext(nc))
        consts = ctx.enter_context(tc.tile_pool(name="consts", bufs=1))
        xp = ctx.enter_context(tc.tile_pool(name="xp", bufs=bufs))
        qp = ctx.enter_context(tc.tile_pool(name="qp", bufs=bufs))
        op = ctx.enter_context(tc.tile_pool(name="op", bufs=bufs))
        vp_bufs = 2 * (stage // pt) if mode == "pipe" else bufs
        vp = ctx.enter_context(tc.tile_pool(name="vp", bufs=vp_bufs))
        pp = ctx.enter_context(
            tc.tile_pool(name="pp", bufs=3 if psum3 else 2, space="PSUM"))
        if psum3:
            ppn2 = ctx.enter_context(tc.tile_pool(name="ppn2", bufs=2,
                                                  space="PSUM"))
        if mode == "full2":
            ppn = ctx.enter_context(tc.tile_pool(name="ppn", bufs=1, space="PSUM"))
        if mode == "pipe":
            ppn = ctx.enter_context(tc.tile_pool(name="ppn", bufs=4, space="PSUM"))

        w0 = consts.tile([128, C], mm_dt)
        nc.sync.dma_start(w0[:], w_d[0])
        w1 = consts.tile([128, C], mm_dt)
        nc.sync.dma_start(w1[:], w_d[1])
        ones = consts.tile([128, 128], BF16)
        nc.vector.memset(ones[:], 1.0)

        def body(mode="full"):
            if dma_only:
                for s in range(npix // stage):
                    c0 = s * stage
                    x0 = xp.tile([128, stage], mm_dt, tag="x0")
                    nc.sync.dma_start(x0[:], x_d[0, :, c0:c0 + stage])
                    x1 = xp.tile([128, stage], mm_dt, tag="x1")
                    getattr(nc, in_eng2).dma_start(x1[:], x_d[1, :, c0:c0 + stage])
                    o0 = op.tile([128, stage], F32, tag="o0")
                    nc.vector.tensor_copy(o0[:, 0:1], x0[:, 0:1])
                    o1 = op.tile([128, stage], F32, tag="o1")
                    nc.vector.tensor_copy(o1[:, 0:1], x1[:, 0:1])
                    getattr(nc, out_eng).dma_start(o_d[0, :, c0:c0 + stage], o0[:])
                    getattr(nc, out_eng).dma_start(o_d[1, :, c0:c0 + stage], o1[:])
                return
            if mode == "pipe":
                nstages = npix // stage
                nt = stage // pt

                def norm_chain(s):
                    """in-DMA + squares + norm matmuls + sqrt + recip for
                    stage s; returns (x0, x1, inv_tiles)."""
                    c0 = s * stage
                    x0 = xp.tile([128, stage], mm_dt, tag="x0")
                    nc.sync.dma_start(x0[:], x_d[0, :, c0:c0 + stage])
                    x1 = xp.tile([128, stage], mm_dt, tag="x1")
                    nc.sync.dma_start(x1[:], x_d[1, :, c0:c0 + stage])
                    q0 = qp.tile([128, stage], BF16, tag="q0")
                    nc.scalar.activation(q0[:], x0[:].bitcast(F32),
                                         mybir.ActivationFunctionType.Square)
                    q1 = qp.tile([128, stage], BF16, tag="q1")
                    nc.scalar.activation(q1[:], x1[:].bitcast(F32),
                                         mybir.ActivationFunctionType.Square)
                    invs = []
                    for t in range(nt):
                        sl = slice(t * pt, (t + 1) * pt)
                        pn = ppn.tile([128, pt], F32, tag="pn")
                        nc.tensor.matmul(pn[:], ones[:], q0[:, sl],
                                         start=True, stop=False)
                        nc.tensor.matmul(pn[:], ones[:], q1[:, sl],
                                         start=False, stop=True)
                        nrm = vp.tile([128, pt], F32, tag="nrm")
                        nc.scalar.activation(nrm[:], pn[:],
                                             mybir.ActivationFunctionType.Sqrt)
                        inv = vp.tile([128, pt], F32, tag="inv")
                        nc.vector.reciprocal_approx_fast(inv[:], nrm[:])
                        invs.append(inv)
                    return x0, x1, invs

                def main_stage(s, x0, x1, invs):
                    c0 = s * stage
                    o0 = op.tile([128, stage], F32, tag="o0")
                    o1 = op.tile([128, stage], F32, tag="o1")
                    for t in range(nt):
                        sl = slice(t * pt, (t + 1) * pt)
                        p0 = pp.tile([128, pt], F32, tag="p0")
                        nc.tensor.matmul(p0[:], w0[:, 0:128], x0[:, sl],
                                         start=True, stop=False)
                        nc.tensor.matmul(p0[:], w1[:, 0:128], x1[:, sl],
                                         start=False, stop=True)
                        p1 = pp.tile([128, pt], F32, tag="p1")
                        nc.tensor.matmul(p1[:], w0[:, 128:256], x0[:, sl],
                                         start=True, stop=False)
                        nc.tensor.matmul(p1[:], w1[:, 128:256], x1[:, sl],
                                         start=False, stop=True)
                        nc.vector.tensor_mul(o0[:, sl], p0[:], invs[t][:])
                        nc.vector.tensor_mul(o1[:, sl], p1[:], invs[t][:])
                    for u0 in range(0, stage, out_split):
                        getattr(nc, out_eng).dma_start(
                            o_d[0, :, c0 + u0:c0 + u0 + out_split],
                            o0[:, u0:u0 + out_split])
                        getattr(nc, out_eng).dma_start(
                            o_d[1, :, c0 + u0:c0 + u0 + out_split],
                            o1[:, u0:u0 + out_split])

                prev = norm_chain(0)
                for s in range(nstages):
                    nxt = norm_chain(s + 1) if s + 1 < nstages else None
                    main_stage(s, *prev)
                    prev = nxt
                return
            if mode == "full2":
                for s in range(npix // stage):
                    c0 = s * stage
                    x0 = xp.tile([128, stage], mm_dt, tag="x0")
                    nc.sync.dma_start(x0[:], x_d[0, :, c0:c0 + stage])
                    x1 = xp.tile([128, stage], mm_dt, tag="x1")
                    nc.sync.dma_start(x1[:], x_d[1, :, c0:c0 + stage])
                    q0 = qp.tile([128, stage], BF16, tag="q0")
                    nc.scalar.activation(q0[:], x0[:].bitcast(F32),
                                         mybir.ActivationFunctionType.Square)
                    q1 = qp.tile([128, stage], BF16, tag="q1")
                    nc.scalar.activation(q1[:], x1[:].bitcast(F32),
                                         mybir.ActivationFunctionType.Square)
                    # stage-granular norm: one 4-bank PSUM strip, then one
                    # sqrt + one reciprocal for the whole stage
                    pn = ppn.tile([128, stage], F32, tag="pn")
                    for t in range(stage // pt):
                        sl = slice(t * pt, (t + 1) * pt)
                        nc.tensor.matmul(pn[:, sl], ones[:], q0[:, sl],
                                         start=True, stop=False)
                        nc.tensor.matmul(pn[:, sl], ones[:], q1[:, sl],
                                         start=False, stop=True)
                    nrm = vp.tile([128, stage], F32, tag="nrm")
                    nc.scalar.activation(nrm[:], pn[:],
                                         mybir.ActivationFunctionType.Sqrt)
                    inv = vp.tile([128, stage], F32, tag="inv")
                    nc.vector.reciprocal_approx_fast(inv[:], nrm[:])
                    o0 = op.tile([128, stage], F32, tag="o0")
                    o1 = op.tile([128, stage], F32, tag="o1")
                    for t in range(stage // pt):
                        sl = slice(t * pt, (t + 1) * pt)
                        p0 = pp.tile([128, pt], F32, tag="p0")
                        nc.tensor.matmul(p0[:], w0[:, 0:128], x0[:, sl],
                                         start=True, stop=False)
                        nc.tensor.matmul(p0[:], w1[:, 0:128], x1[:, sl],
                                         start=False, stop=True)
                        p1 = pp.tile([128, pt], F32, tag="p1")
                        nc.tensor.matmul(p1[:], w0[:, 128:256], x0[:, sl],
                                         start=True, stop=False)
                        nc.tensor.matmul(p1[:], w1[:, 128:256], x1[:, sl],
                                         start=False, stop=True)
                        nc.vector.tensor_mul(o0[:, sl], p0[:], inv[:, sl])
                        nc.vector.tensor_mul(o1[:, sl], p1[:], inv[:, sl])
                    for u0 in range(0, stage, out_split):
                        getattr(nc, out_eng).dma_start(
                            o_d[0, :, c0 + u0:c0 + u0 + out_split],
                            o0[:, u0:u0 + out_split])
                        getattr(nc, out_eng).dma_start(
                            o_d[1, :, c0 + u0:c0 + u0 + out_split],
                            o1[:, u0:u0 + out_split])
                return
            if mode == "fullm":
                # merged-DMA variant: one 2MB in-DMA and one 2MB out-DMA per
                # 2048-px stage (both d-chunks in a single [128, 2*stage]
                # tile) — bigger transfers, same pipeline granularity.
                for s in range(npix // stage):
                    c0 = s * stage
                    xt = xp.tile([128, 2 * stage], mm_dt, tag="xt")
                    nc.sync.dma_start(
                        xt[:].rearrange("p (c n) -> p c n", c=2),
                        x_d[:, :, c0:c0 + stage].rearrange("c p n -> p c n"))
                    x0 = xt[:, 0:stage]
                    x1 = xt[:, stage:2 * stage]
                    q0 = qp.tile([128, stage], BF16, tag="q0")
                    nc.scalar.activation(q0[:], x0.bitcast(F32),
                                         mybir.ActivationFunctionType.Square)
                    q1 = qp.tile([128, stage], BF16, tag="q1")
                    nc.scalar.activation(q1[:], x1.bitcast(F32),
                                         mybir.ActivationFunctionType.Square)
                    ot = op.tile([128, 2 * stage], F32, tag="ot")
                    for t in range(stage // pt):
                        sl = slice(t * pt, (t + 1) * pt)
                        sl1 = slice(stage + t * pt, stage + (t + 1) * pt)
                        pn = pp.tile([128, pt], F32, tag="pn")
                        nc.tensor.matmul(pn[:], ones[:], q0[:, sl],
                                         start=True, stop=False)
                        nc.tensor.matmul(pn[:], ones[:], q1[:, sl],
                                         start=False, stop=True)
                        p0 = pp.tile([128, pt], F32, tag="p0")
                        nc.tensor.matmul(p0[:], w0[:, 0:128], x0[:, sl],
                                         start=True, stop=False)
                        nc.tensor.matmul(p0[:], w1[:, 0:128], x1[:, sl],
                                         start=False, stop=True)
                        p1 = pp.tile([128, pt], F32, tag="p1")
                        nc.tensor.matmul(p1[:], w0[:, 128:256], x0[:, sl],
                                         start=True, stop=False)
                        nc.tensor.matmul(p1[:], w1[:, 128:256], x1[:, sl],
                                         start=False, stop=True)
                        nrm = vp.tile([128, pt], F32, tag="nrm")
                        nc.scalar.activation(nrm[:], pn[:],
                                             mybir.ActivationFunctionType.Sqrt)
                        inv = vp.tile([128, pt], F32, tag="inv")
                        nc.vector.reciprocal_approx_fast(inv[:], nrm[:])
                        nc.vector.tensor_mul(ot[:, sl], p0[:], inv[:])
                        nc.vector.tensor_mul(ot[:, sl1], p1[:], inv[:])
                    getattr(nc, out_eng).dma_start(
                        o_d[:, :, c0:c0 + stage].rearrange("c p n -> p c n"),
                        ot[:].rearrange("p (c n) -> p c n", c=2))
                return
            do_sq = mode in ("full", "mmq", "mmqn", "mmqns")
            do_pn = mode in ("full", "mmqn", "mmqns")
            do_sqrt = mode in ("full", "mmqns")
            do_recip = mode == "full"
            for s in range(npix // stage):
                c0 = s * stage
                x0 = xp.tile([128, stage], mm_dt, tag="x0")
                nc.sync.dma_start(x0[:], x_d[0, :, c0:c0 + stage])
                x1 = xp.tile([128, stage], mm_dt, tag="x1")
                getattr(nc, in_eng2).dma_start(x1[:], x_d[1, :, c0:c0 + stage])
                if do_sq:
                    q0 = qp.tile([128, stage], BF16, tag="q0")
                    nc.scalar.activation(q0[:], x0[:].bitcast(F32),
                                         mybir.ActivationFunctionType.Square)
                    q1 = qp.tile([128, stage], BF16, tag="q1")
                    nc.scalar.activation(q1[:], x1[:].bitcast(F32),
                                         mybir.ActivationFunctionType.Square)
                o0 = op.tile([128, stage], F32, tag="o0")
                o1 = op.tile([128, stage], F32, tag="o1")
                for t in range(stage // pt):
                    sl = slice(t * pt, (t + 1) * pt)
                    if do_pn:
                        pn = (ppn2 if psum3 else pp).tile([128, pt], F32,
                                                          tag="pn")
                        nc.tensor.matmul(pn[:], ones[:], q0[:, sl],
                                         start=True, stop=False)
                        nc.tensor.matmul(pn[:], ones[:], q1[:, sl],
                                         start=False, stop=True)
                    p0 = pp.tile([128, pt], F32, tag="p0")
                    nc.tensor.matmul(p0[:], w0[:, 0:128], x0[:, sl],
                                     start=True, stop=False)
                    nc.tensor.matmul(p0[:], w1[:, 0:128], x1[:, sl],
                                     start=False, stop=True)
                    p1 = pp.tile([128, pt], F32, tag="p1")
                    nc.tensor.matmul(p1[:], w0[:, 128:256], x0[:, sl],
                                     start=True, stop=False)
                    nc.tensor.matmul(p1[:], w1[:, 128:256], x1[:, sl],
                                     start=False, stop=True)
                    if do_sqrt:
                        nrm = vp.tile([128, pt], F32, tag="nrm")
                        nc.scalar.activation(nrm[:], pn[:],
                                             mybir.ActivationFunctionType.Sqrt)
                    if do_recip:
                        inv = vp.tile([128, pt], F32, tag="inv")
                        nc.vector.reciprocal_approx_fast(inv[:], nrm[:])
                        nc.vector.tensor_mul(o0[:, sl], p0[:], inv[:])
                        nc.vector.tensor_mul(o1[:, sl], p1[:], inv[:])
                    elif do_sqrt:
                        nc.vector.tensor_mul(o0[:, sl], p0[:], nrm[:])
                        nc.vector.tensor_mul(o1[:, sl], p1[:], nrm[:])
                    else:
                        nc.vector.tensor_copy(o0[:, sl], p0[:])
                        nc.vector.tensor_copy(o1[:, sl], p1[:])
                for u0 in range(0, stage, out_split):
                    getattr(nc, out_eng).dma_start(
                        o_d[0, :, c0 + u0:c0 + u0 + out_split],
                        o0[:, u0:u0 + out_split])
                    getattr(nc, out_eng).dma_start(
                        o_d[1, :, c0 + u0:c0 + u0 + out_split],
                        o1[:, u0:u0 + out_split])

        if repeat == 1:
            body(mode)
        else:
            with tc.For_i(0, repeat, 1, staggered_reset=staggered):
                body(mode)

    nc.compile()
    return nc


def _get_prog():
    key = "main"
    if key not in _CACHE:
        _CACHE[key] = build()
    return _CACHE[key]


def prep_inputs(x, weights, scale):
    """Host-side prep: shard x spatially, fold norm+scale into transposed
    weights. Returns in_maps for the 8 cores."""
    x = np.ascontiguousarray(np.asarray(x, dtype=np.float32))
    weights = np.asarray(weights, dtype=np.float32)
    scale = np.asarray(scale, dtype=np.float32)

    wnorm = np.sqrt((weights * weights).sum(axis=1))
    sfold = (RANGE_EXTENDER * scale) / np.maximum(wnorm, EPS)
    wT = np.ascontiguousarray((weights * sfold[:, None]).T.astype(np.float32))
    wT = wT.reshape(2, 128, C)

    in_maps = []
    for k in range(N_CORES):
        xl = np.ascontiguousarray(x[:, k * HL:(k + 1) * HL, :])
        in_maps.append({"x": xl.reshape(2, 128, NPIX), "wt": wT})
    return in_maps


def gather_output(results):
    outs = [res["out"].reshape(C, HL, W) for res in results]
    return np.concatenate(outs, axis=1)


def kernel(x, weights, scale):
    in_maps = prep_inputs(x, weights, scale)
    nc = _get_prog()
    res = run_bass_kernel_spmd(nc, in_maps, core_ids=list(range(N_CORES)))
    return gather_output(res.results)
